# revision 3
# baseline (speedup 1.0000x reference)
"""Trainium2 Bass kernel: MeanHinAggregator (GNN message passing).

Reference computation (per batch-head element bh):
    z_r  = mean_n(x_neigh_r[bh, n, :]) @ w_neigh_r          (r = 0, 1)
    out  = relu(concat(x_self[bh] @ w_self, (z0 + z1) / 2) + b)

Strategy (pure data parallel over 8 NeuronCores, batch axis sharded):
  * The 2e-2 relative-error budget admits a bf16 datapath.  All activations
    and weights are cast to bf16 on the host during the shard step, halving
    the HBM traffic that dominates this memory-bound problem.  The output
    is stored as bf16 and upcast on the host (rel-err stays ~2.5e-3).
  * Per core: B_shard=128, H=10 -> 1280 rows, processed in 10 groups of 128.
    xn0 / xn1 / x_self are host-packed into ONE row-major tensor
    [1280, 4096+4096+128] so each group is a single 2.03 MiB DMA
    (16.25 KiB contiguous per partition) - large transfers keep the
    HWDGE queues at line rate and avoid 256 B x_self descriptors.
    Groups alternate between the SP and ACT HWDGE rings; output stores
    ride the ACT ring right after their ReLU with no cross-engine stall.
  * The first and last groups are instead split into five DMAs across
    both rings (two halves per neighbour tensor + x_self), so the first
    fold starts ~4 us earlier and the post-last-DMA serial tail is short.
  * Mean over the 32 neighbour slices: in-place bf16 strided adds on the
    Vector engine (bf16 tensor_tensor hits the 2x_1P DVE perf mode) fold
    32 slices to 2, then two accumulating transposing matmuls
    (lhsT = slice, rhs = identity) finish the sum while transposing into
    the [f, bh] layout the projection needs as lhsT.  Keeping the last
    fold level on the PE shortens the tail, which is pure serial latency.
  * Projection: out[bh, d] = sumT.T @ w with the 1/(N*NR) scaling folded
    into host-prescaled bf16 copies of w_neigh_*.  Bias is added with a
    K=1 matmul (lhsT = ones row, rhs = bias row) accumulating into PSUM,
    ordered so the data matmul that depends on the last-arriving tensor
    is the only one on the critical tail.  PSUM -> SBUF copies are split
    per 128-column block on the Scalar engine.
"""

import numpy as np
import ml_dtypes

import concourse.bacc as bacc
import concourse.bass as bass
import concourse.tile as tile
from concourse import bass_utils, mybir
from concourse._compat import with_exitstack

B, H, N, F = 1024, 10, 32, 128
HALF = 128
D = 2 * HALF
NR = 2
NCORES = 8
BSH = B // NCORES        # 128 batch rows per core
BH = BSH * H             # 1280 (bh rows per core)
GROUP = 128              # bh rows per group
NG = BH // GROUP         # 10 groups
NF = N * F               # 4096
PACK = 2 * NF + F        # 8320 packed row: [xn0 | xn1 | xs]
LOOKAHEAD = 3            # groups of DMA prefetch beyond the current one
F32 = mybir.dt.float32
BF16 = mybir.dt.bfloat16
BF16NP = np.dtype(ml_dtypes.bfloat16)
RELU = mybir.ActivationFunctionType.Relu
COPY = mybir.ActivationFunctionType.Copy


@with_exitstack
def _tile_kernel(ctx, tc, outs, ins, ngroups):
    nc = tc.nc
    xpk, w_s, w0, w1, bvec, ident_d, ones_d = ins
    (out_d,) = outs

    const = ctx.enter_context(tc.tile_pool(name="const", bufs=1))
    xpool = ctx.enter_context(tc.tile_pool(name="xp", bufs=LOOKAHEAD + 1))
    spool = ctx.enter_context(tc.tile_pool(name="sp", bufs=3))
    opool = ctx.enter_context(tc.tile_pool(name="op", bufs=3))
    ppool = ctx.enter_context(tc.tile_pool(name="ps", bufs=2, space="PSUM"))
    pout = ctx.enter_context(tc.tile_pool(name="po", bufs=2, space="PSUM"))

    split = {0, ngroups - 1}  # groups DMA'd in 5 pieces across both rings

    def issue_loads(g):
        r = slice(g * GROUP, (g + 1) * GROUP)
        t = xpool.tile([128, PACK], BF16, tag="t")
        if g in split:
            nc.sync.dma_start(t[:, 0:2048], xpk[r, 0:2048])
            nc.scalar.dma_start(t[:, NF:NF + 2048], xpk[r, NF:NF + 2048])
            nc.sync.dma_start(t[:, 2048:NF], xpk[r, 2048:NF])
            nc.scalar.dma_start(t[:, NF + 2048:2 * NF],
                                xpk[r, NF + 2048:2 * NF])
            nc.sync.dma_start(t[:, 2 * NF:PACK], xpk[r, 2 * NF:PACK])
        else:
            eng = nc.sync if g % 2 else nc.scalar
            eng.dma_start(t[:], xpk[r, :])
        return t

    pending = [issue_loads(0)]

    ident = const.tile([128, 128], BF16, tag="ident")
    nc.sync.dma_start(ident[:], ident_d[:])
    wS_t = const.tile([128, HALF], BF16, tag="wS")
    nc.sync.dma_start(wS_t[:], w_s[:])
    w0_t = const.tile([128, HALF], BF16, tag="w0")
    nc.sync.dma_start(w0_t[:], w0[:])
    w1_t = const.tile([128, HALF], BF16, tag="w1")
    nc.sync.dma_start(w1_t[:], w1[:])
    b_t = const.tile([1, D], BF16, tag="b")
    nc.sync.dma_start(b_t[:], bvec[:])
    ones_t = const.tile([1, 128], BF16, tag="ones")
    nc.sync.dma_start(ones_t[:], ones_d[:])

    for g in range(1, min(LOOKAHEAD, ngroups)):
        pending.append(issue_loads(g))

    def fold(t, base, nslices):
        """In-place binary-tree fold of `nslices` F-wide slices at column
        `base` down to 2 slices; returns their column offsets."""
        lv = nslices // 2
        while lv >= 2:
            nc.vector.tensor_add(t[:, base:base + lv * F],
                                 t[:, base:base + lv * F],
                                 t[:, base + lv * F:base + 2 * lv * F])
            lv //= 2
        return (base, base + F)

    for g in range(ngroups):
        r = slice(g * GROUP, (g + 1) * GROUP)
        t = pending.pop(0)
        if g + LOOKAHEAD < ngroups:
            pending.append(issue_loads(g + LOOKAHEAD))

        # Fold each neighbour tensor's 32 slices to 2 (or 2x 16 -> 2+2 for
        # the split ramp/tail groups, whose halves arrive independently).
        if g in split:
            sl0 = fold(t, 0, 16) + fold(t, 2048, 16)
            sl1 = fold(t, NF, 16) + fold(t, NF + 2048, 16)
        else:
            sl0 = fold(t, 0, 32)
            sl1 = fold(t, NF, 32)

        # pacc[:, 0:128] = sum_n x_n0 (as [f, bh]), [:, 128:256] = sum_n
        # x_n1, [:, 256:384] = x_self; accumulating transposing matmuls.
        pacc = ppool.tile([128, 3 * 128], F32, tag="pacc")
        nc.tensor.matmul(pacc[:, 256:384], t[:, 2 * NF:PACK], ident[:],
                         start=True, stop=True)
        for i, c in enumerate(sl0):
            nc.tensor.matmul(pacc[:, 0:128], t[:, c:c + F], ident[:],
                             start=(i == 0), stop=(i == len(sl0) - 1))
        for i, c in enumerate(sl1):
            nc.tensor.matmul(pacc[:, 128:256], t[:, c:c + F], ident[:],
                             start=(i == 0), stop=(i == len(sl1) - 1))

        # PSUM -> SBUF (bf16) in per-block copies so the self/t0 projections
        # don't wait for t1's transposes.
        sacc = spool.tile([128, 3 * 128], BF16, tag="sacc")
        nc.scalar.activation(sacc[:, 256:384], pacc[:, 256:384], COPY)
        nc.scalar.activation(sacc[:, 0:128], pacc[:, 0:128], COPY)
        nc.scalar.activation(sacc[:, 128:256], pacc[:, 128:256], COPY)

        # Projection: out[bh, d]; bias broadcast via K=1 matmuls, ordered so
        # the t1-dependent matmul is the only one on the critical tail.
        po = pout.tile([128, D], F32, tag="po")
        nc.tensor.matmul(po[:, 0:HALF], ones_t[:], b_t[:, 0:HALF],
                         start=True, stop=False)
        nc.tensor.matmul(po[:, 0:HALF], sacc[:, 256:384], wS_t[:],
                         start=False, stop=True)
        nc.tensor.matmul(po[:, HALF:D], ones_t[:], b_t[:, HALF:D],
                         start=True, stop=False)
        nc.tensor.matmul(po[:, HALF:D], sacc[:, 0:128], w0_t[:],
                         start=False, stop=False)
        nc.tensor.matmul(po[:, HALF:D], sacc[:, 128:256], w1_t[:],
                         start=False, stop=True)

        ob = opool.tile([128, D], BF16, tag="ob")
        nc.scalar.activation(ob[:], po[:], RELU)
        nc.scalar.dma_start(out_d[r, :], ob[:])


def build_nc(ngroups=NG):
    bh = ngroups * GROUP
    nc = bacc.Bacc("TRN2", target_bir_lowering=False, debug=False)
    xpk = nc.dram_tensor("xpk", [bh, PACK], BF16, kind="ExternalInput")
    w_s = nc.dram_tensor("w_s", [F, HALF], BF16, kind="ExternalInput")
    w0 = nc.dram_tensor("w0", [F, HALF], BF16, kind="ExternalInput")
    w1 = nc.dram_tensor("w1", [F, HALF], BF16, kind="ExternalInput")
    bvec = nc.dram_tensor("bvec", [1, D], BF16, kind="ExternalInput")
    ident_d = nc.dram_tensor("ident", [128, 128], BF16, kind="ExternalInput")
    ones_d = nc.dram_tensor("ones", [1, 128], BF16, kind="ExternalInput")
    out = nc.dram_tensor("out", [bh, D], BF16, kind="ExternalOutput")

    ins = [t.ap() for t in (xpk, w_s, w0, w1, bvec, ident_d, ones_d)]
    with nc.allow_low_precision("2e-2 rel-err budget admits bf16 datapath"):
        with tile.TileContext(nc) as tc:
            _tile_kernel(tc, [out.ap()], ins, ngroups)
    nc.compile()
    return nc


def make_in_maps(x_self, x_neigh_0, x_neigh_1, w_self, w_neigh_0, w_neigh_1, b):
    """Shard full inputs into per-core input maps (batch axis, 8 ways).

    All operands are cast to bf16 here (host side): the 2e-2 tolerance
    admits it and it halves the HBM traffic of this memory-bound kernel.
    xn0/xn1/xs are packed into one row-major tensor so each row group is
    a single large DMA.
    """
    xs16 = np.asarray(x_self, dtype=np.float32).astype(BF16NP)
    xn0_16 = np.asarray(x_neigh_0, dtype=np.float32).astype(BF16NP)
    xn1_16 = np.asarray(x_neigh_1, dtype=np.float32).astype(BF16NP)
    scale = np.float32(1.0 / (N * NR))
    w_s = np.asarray(w_self, dtype=np.float32).astype(BF16NP)
    w0 = (np.asarray(w_neigh_0, dtype=np.float32) * scale).astype(BF16NP)
    w1 = (np.asarray(w_neigh_1, dtype=np.float32) * scale).astype(BF16NP)
    bvec = np.asarray(b, dtype=np.float32).reshape(1, D).astype(BF16NP)
    ident = np.eye(128, dtype=np.float32).astype(BF16NP)
    ones = np.ones((1, 128), dtype=np.float32).astype(BF16NP)

    xpk = np.empty((B * H, PACK), dtype=BF16NP)
    xpk[:, 0:NF] = xn0_16.reshape(B * H, NF)
    xpk[:, NF:2 * NF] = xn1_16.reshape(B * H, NF)
    xpk[:, 2 * NF:PACK] = xs16.reshape(B * H, F)

    in_maps = []
    for c in range(NCORES):
        rs = slice(c * BH, (c + 1) * BH)
        in_maps.append({
            "xpk": np.ascontiguousarray(xpk[rs]),
            "w_s": w_s, "w0": w0, "w1": w1, "bvec": bvec,
            "ident": ident, "ones": ones,
        })
    return in_maps


_NC_CACHE = None


def kernel(x_self, x_neigh_0, x_neigh_1, w_self, w_neigh_0, w_neigh_1, b):
    global _NC_CACHE
    if _NC_CACHE is None:
        _NC_CACHE = build_nc()
    in_maps = make_in_maps(x_self, x_neigh_0, x_neigh_1,
                           w_self, w_neigh_0, w_neigh_1, b)
    res = bass_utils.run_bass_kernel_spmd(
        _NC_CACHE, in_maps, core_ids=list(range(NCORES)))
    out = np.concatenate([r["out"] for r in res.results], axis=0)
    return out.astype(np.float32).reshape(B, H, D)


# revision 9
# speedup vs baseline: 1.0582x; 1.0582x over previous
"""Trainium2 Bass kernel: MeanHinAggregator (GNN message passing).

Reference computation (per batch-head element bh):
    z_r  = mean_n(x_neigh_r[bh, n, :]) @ w_neigh_r          (r = 0, 1)
    out  = relu(concat(x_self[bh] @ w_self, (z0 + z1) / 2) + b)

Strategy (pure data parallel over 8 NeuronCores, batch axis sharded):
  * The 2e-2 relative-error budget admits a bf16 datapath.  All activations
    and weights are cast to bf16 on the host during the shard step, halving
    the HBM traffic that dominates this memory-bound problem.  The output
    is stored as bf16 and upcast on the host (rel-err stays ~2.5e-3).
  * Per core: B_shard=128, H=10 -> 1280 rows, processed in 10 groups of 128.
    xn0 / x_self / xn1 are host-packed into ONE row-major tensor
    [1280, 4096+128+4096] so each group is exactly two ~1 MiB DMAs:
    xn0+xs on the SP HWDGE ring, xn1 on the ACT ring (contiguous 8.25 KiB
    per partition each - no 256 B x_self descriptor storms).  Each
    engine dispatches its ring's load every iteration (3 groups ahead),
    keeping both rings continuously fed; output stores ride the ACT ring
    right after their ReLU with no cross-engine stall.
  * The first and last groups are split into two DMAs per ring, so the
    first fold starts ~3 us earlier and the post-last-DMA serial tail is
    shorter.
  * Mean over the 32 neighbour slices: in-place bf16 strided adds on the
    Vector engine (bf16 tensor_tensor hits the 2x_1P DVE perf mode) fold
    32 slices to 2, then two accumulating transposing matmuls
    (lhsT = slice, rhs = identity) finish the sum while transposing into
    the [f, bh] layout the projection needs as lhsT.  Keeping the last
    fold level on the PE shortens the tail, which is pure serial latency.
  * Projection: out[bh, d] = sumT.T @ w with the 1/(N*NR) scaling folded
    into host-prescaled bf16 copies of w_neigh_*.  Bias is added with a
    K=1 matmul (lhsT = ones row, rhs = bias row) accumulating into PSUM,
    ordered so the data matmul that depends on the last-arriving tensor
    is the only one on the critical tail.  PSUM -> SBUF copies are split
    per 128-column block on the Scalar engine.
"""

import numpy as np
import ml_dtypes

import concourse.bacc as bacc
import concourse.bass as bass
import concourse.tile as tile
from concourse import bass_utils, mybir
from concourse._compat import with_exitstack

B, H, N, F = 1024, 10, 32, 128
HALF = 128
D = 2 * HALF
NR = 2
NCORES = 8
BSH = B // NCORES        # 128 batch rows per core
BH = BSH * H             # 1280 (bh rows per core)
GROUP = 128              # bh rows per group
NG = BH // GROUP         # 10 groups
NF = N * F               # 4096
PACK = 2 * NF + F        # 8320 packed row: [xn0 | xs | xn1]
T1B = NF + F             # 4224: column where xn1 starts in a packed row
LOOKAHEAD = 3            # groups of DMA prefetch beyond the current one
F32 = mybir.dt.float32
BF16 = mybir.dt.bfloat16
BF16NP = np.dtype(ml_dtypes.bfloat16)
RELU = mybir.ActivationFunctionType.Relu
COPY = mybir.ActivationFunctionType.Copy


@with_exitstack
def _tile_kernel(ctx, tc, outs, ins, ngroups):
    nc = tc.nc
    xpk, w_s, w0, w1, bvec, ident_d, ones_d = ins
    (out_d,) = outs

    const = ctx.enter_context(tc.tile_pool(name="const", bufs=1))
    xpool = ctx.enter_context(tc.tile_pool(name="xp", bufs=LOOKAHEAD + 1))
    spool = ctx.enter_context(tc.tile_pool(name="sp", bufs=3))
    opool = ctx.enter_context(tc.tile_pool(name="op", bufs=3))
    ppool = ctx.enter_context(tc.tile_pool(name="ps", bufs=2, space="PSUM"))
    pout = ctx.enter_context(tc.tile_pool(name="po", bufs=2, space="PSUM"))

    split = {0, ngroups - 1}  # groups DMA'd in 4 pieces for short ramp/tail

    def issue_loads(g):
        """Every group rides both HWDGE rings: xn0+xs on SP, xn1 on ACT
        (stores join ACT).  Ramp/tail groups split each half again so the
        first fold starts earlier / the serial tail is shorter."""
        r = slice(g * GROUP, (g + 1) * GROUP)
        t = xpool.tile([128, PACK], BF16, tag="t")
        if g in split:
            nc.sync.dma_start(t[:, 0:2048], xpk[r, 0:2048])
            nc.scalar.dma_start(t[:, T1B:T1B + 2048], xpk[r, T1B:T1B + 2048])
            nc.sync.dma_start(t[:, 2048:T1B], xpk[r, 2048:T1B])
            nc.scalar.dma_start(t[:, T1B + 2048:PACK],
                                xpk[r, T1B + 2048:PACK])
        else:
            nc.sync.dma_start(t[:, 0:T1B], xpk[r, 0:T1B])
            nc.scalar.dma_start(t[:, T1B:PACK], xpk[r, T1B:PACK])
        return t

    pending = [issue_loads(0)]

    ident = const.tile([128, 128], BF16, tag="ident")
    nc.sync.dma_start(ident[:], ident_d[:])
    wS_t = const.tile([128, HALF], BF16, tag="wS")
    nc.sync.dma_start(wS_t[:], w_s[:])
    w0_t = const.tile([128, HALF], BF16, tag="w0")
    nc.sync.dma_start(w0_t[:], w0[:])
    w1_t = const.tile([128, HALF], BF16, tag="w1")
    nc.sync.dma_start(w1_t[:], w1[:])
    b_t = const.tile([1, D], BF16, tag="b")
    nc.sync.dma_start(b_t[:], bvec[:])
    ones_t = const.tile([1, 128], BF16, tag="ones")
    nc.sync.dma_start(ones_t[:], ones_d[:])

    for g in range(1, min(LOOKAHEAD, ngroups)):
        pending.append(issue_loads(g))

    def fold(t, base, nslices):
        """In-place binary-tree fold of `nslices` F-wide slices at column
        `base` down to 2 slices; returns their column offsets."""
        lv = nslices // 2
        while lv >= 2:
            nc.vector.tensor_add(t[:, base:base + lv * F],
                                 t[:, base:base + lv * F],
                                 t[:, base + lv * F:base + 2 * lv * F])
            lv //= 2
        return (base, base + F)

    for g in range(ngroups):
        r = slice(g * GROUP, (g + 1) * GROUP)
        t = pending.pop(0)
        if g + LOOKAHEAD < ngroups:
            pending.append(issue_loads(g + LOOKAHEAD))

        # Fold each neighbour tensor's 32 slices to 2 (or 2x 16 -> 2+2 for
        # the split ramp/tail groups, whose halves arrive independently).
        if g in split:
            sl0 = fold(t, 0, 16) + fold(t, 2048, 16)
            sl1 = fold(t, T1B, 16) + fold(t, T1B + 2048, 16)
        else:
            sl0 = fold(t, 0, 32)
            sl1 = fold(t, T1B, 32)

        # pacc[:, 0:128] = sum_n x_n0 (as [f, bh]), [:, 128:256] = sum_n
        # x_n1, [:, 256:384] = x_self; accumulating transposing matmuls.
        pacc = ppool.tile([128, 3 * 128], F32, tag="pacc")
        nc.tensor.matmul(pacc[:, 256:384], t[:, NF:T1B], ident[:],
                         start=True, stop=True)
        for i, c in enumerate(sl0):
            nc.tensor.matmul(pacc[:, 0:128], t[:, c:c + F], ident[:],
                             start=(i == 0), stop=(i == len(sl0) - 1))
        for i, c in enumerate(sl1):
            nc.tensor.matmul(pacc[:, 128:256], t[:, c:c + F], ident[:],
                             start=(i == 0), stop=(i == len(sl1) - 1))

        # PSUM -> SBUF (bf16) in per-block copies so the self/t0 projections
        # don't wait for t1's transposes.
        sacc = spool.tile([128, 3 * 128], BF16, tag="sacc")
        nc.scalar.activation(sacc[:, 256:384], pacc[:, 256:384], COPY)
        nc.scalar.activation(sacc[:, 0:128], pacc[:, 0:128], COPY)
        nc.scalar.activation(sacc[:, 128:256], pacc[:, 128:256], COPY)

        # Projection: out[bh, d]; bias broadcast via K=1 matmuls, ordered so
        # the t1-dependent matmul is the only one on the critical tail.
        po = pout.tile([128, D], F32, tag="po")
        nc.tensor.matmul(po[:, 0:HALF], ones_t[:], b_t[:, 0:HALF],
                         start=True, stop=False)
        nc.tensor.matmul(po[:, 0:HALF], sacc[:, 256:384], wS_t[:],
                         start=False, stop=True)
        nc.tensor.matmul(po[:, HALF:D], ones_t[:], b_t[:, HALF:D],
                         start=True, stop=False)
        nc.tensor.matmul(po[:, HALF:D], sacc[:, 0:128], w0_t[:],
                         start=False, stop=False)
        nc.tensor.matmul(po[:, HALF:D], sacc[:, 128:256], w1_t[:],
                         start=False, stop=True)

        ob = opool.tile([128, D], BF16, tag="ob")
        nc.scalar.activation(ob[:], po[:], RELU)
        nc.scalar.dma_start(out_d[r, :], ob[:])


def build_nc(ngroups=NG):
    bh = ngroups * GROUP
    nc = bacc.Bacc("TRN2", target_bir_lowering=False, debug=False)
    xpk = nc.dram_tensor("xpk", [bh, PACK], BF16, kind="ExternalInput")
    w_s = nc.dram_tensor("w_s", [F, HALF], BF16, kind="ExternalInput")
    w0 = nc.dram_tensor("w0", [F, HALF], BF16, kind="ExternalInput")
    w1 = nc.dram_tensor("w1", [F, HALF], BF16, kind="ExternalInput")
    bvec = nc.dram_tensor("bvec", [1, D], BF16, kind="ExternalInput")
    ident_d = nc.dram_tensor("ident", [128, 128], BF16, kind="ExternalInput")
    ones_d = nc.dram_tensor("ones", [1, 128], BF16, kind="ExternalInput")
    out = nc.dram_tensor("out", [bh, D], BF16, kind="ExternalOutput")

    ins = [t.ap() for t in (xpk, w_s, w0, w1, bvec, ident_d, ones_d)]
    with nc.allow_low_precision("2e-2 rel-err budget admits bf16 datapath"):
        with tile.TileContext(nc) as tc:
            _tile_kernel(tc, [out.ap()], ins, ngroups)
    nc.compile()
    return nc


def make_in_maps(x_self, x_neigh_0, x_neigh_1, w_self, w_neigh_0, w_neigh_1, b):
    """Shard full inputs into per-core input maps (batch axis, 8 ways).

    All operands are cast to bf16 here (host side): the 2e-2 tolerance
    admits it and it halves the HBM traffic of this memory-bound kernel.
    xn0/xn1/xs are packed into one row-major tensor so each row group is
    a single large DMA.
    """
    xs16 = np.asarray(x_self, dtype=np.float32).astype(BF16NP)
    xn0_16 = np.asarray(x_neigh_0, dtype=np.float32).astype(BF16NP)
    xn1_16 = np.asarray(x_neigh_1, dtype=np.float32).astype(BF16NP)
    scale = np.float32(1.0 / (N * NR))
    w_s = np.asarray(w_self, dtype=np.float32).astype(BF16NP)
    w0 = (np.asarray(w_neigh_0, dtype=np.float32) * scale).astype(BF16NP)
    w1 = (np.asarray(w_neigh_1, dtype=np.float32) * scale).astype(BF16NP)
    bvec = np.asarray(b, dtype=np.float32).reshape(1, D).astype(BF16NP)
    ident = np.eye(128, dtype=np.float32).astype(BF16NP)
    ones = np.ones((1, 128), dtype=np.float32).astype(BF16NP)

    xpk = np.empty((B * H, PACK), dtype=BF16NP)
    xpk[:, 0:NF] = xn0_16.reshape(B * H, NF)
    xpk[:, NF:T1B] = xs16.reshape(B * H, F)
    xpk[:, T1B:PACK] = xn1_16.reshape(B * H, NF)

    in_maps = []
    for c in range(NCORES):
        rs = slice(c * BH, (c + 1) * BH)
        in_maps.append({
            "xpk": np.ascontiguousarray(xpk[rs]),
            "w_s": w_s, "w0": w0, "w1": w1, "bvec": bvec,
            "ident": ident, "ones": ones,
        })
    return in_maps


_NC_CACHE = None


def kernel(x_self, x_neigh_0, x_neigh_1, w_self, w_neigh_0, w_neigh_1, b):
    global _NC_CACHE
    if _NC_CACHE is None:
        _NC_CACHE = build_nc()
    in_maps = make_in_maps(x_self, x_neigh_0, x_neigh_1,
                           w_self, w_neigh_0, w_neigh_1, b)
    res = bass_utils.run_bass_kernel_spmd(
        _NC_CACHE, in_maps, core_ids=list(range(NCORES)))
    out = np.concatenate([r["out"] for r in res.results], axis=0)
    return out.astype(np.float32).reshape(B, H, D)


# revision 12
# speedup vs baseline: 1.0876x; 1.0277x over previous
"""Trainium2 Bass kernel: MeanHinAggregator (GNN message passing).

Reference computation (per batch-head element bh):
    z_r  = mean_n(x_neigh_r[bh, n, :]) @ w_neigh_r          (r = 0, 1)
    out  = relu(concat(x_self[bh] @ w_self, (z0 + z1) / 2) + b)

Strategy (pure data parallel over 8 NeuronCores, batch axis sharded):
  * The 2e-2 relative-error budget admits a bf16 datapath.  All activations
    and weights are cast to bf16 on the host during the shard step, halving
    the HBM traffic that dominates this memory-bound problem.  The output
    is stored as bf16 and upcast on the host (rel-err stays ~2.5e-3).
  * Per core: B_shard=128, H=10 -> 1280 rows, processed in 10 groups of 128.
    xn0 / x_self / xn1 are host-packed into ONE row-major tensor
    [1280, 4096+128+4096] so each group is exactly two ~1 MiB DMAs:
    xn0+xs on the SP HWDGE ring, xn1 on the ACT ring (contiguous 8.25 KiB
    per partition each - no 256 B x_self descriptor storms).  Each
    engine dispatches its ring's load every iteration (3 groups ahead),
    keeping both rings continuously fed; output stores ride the ACT ring
    right after their ReLU with no cross-engine stall.
  * The first and last groups are split into two DMAs per ring, so the
    first fold starts ~3 us earlier and the post-last-DMA serial tail is
    shorter.
  * Mean over the 32 neighbour slices: in-place bf16 strided adds on the
    Vector engine (bf16 tensor_tensor hits the 2x_1P DVE perf mode) fold
    32 slices to 2, then two accumulating transposing matmuls
    (lhsT = slice, rhs = identity) finish the sum while transposing into
    the [f, bh] layout the projection needs as lhsT.  Keeping the last
    fold level on the PE shortens the tail, which is pure serial latency.
  * Projection: out[bh, d] = sumT.T @ w with the 1/(N*NR) scaling folded
    into host-prescaled bf16 copies of w_neigh_*.  Bias is added with a
    K=1 matmul (lhsT = ones row, rhs = bias row) accumulating into PSUM,
    ordered so the data matmul that depends on the last-arriving tensor
    is the only one on the critical tail.  PSUM -> SBUF copies are split
    per 128-column block on the Scalar engine.
"""

import numpy as np
import ml_dtypes

import concourse.bacc as bacc
import concourse.bass as bass
import concourse.tile as tile
from concourse import bass_utils, mybir
from concourse._compat import with_exitstack

B, H, N, F = 1024, 10, 32, 128
HALF = 128
D = 2 * HALF
NR = 2
NCORES = 8
BSH = B // NCORES        # 128 batch rows per core
BH = BSH * H             # 1280 (bh rows per core)
GROUP = 128              # bh rows per group
NG = BH // GROUP         # 10 groups
NF = N * F               # 4096
PACK = 2 * NF + F        # 8320 packed row: [xn0 | xs | xn1]
T1B = NF + F             # 4224: column where xn1 starts in a packed row
LOOKAHEAD = 4            # groups of DMA prefetch beyond the current one
F32 = mybir.dt.float32
BF16 = mybir.dt.bfloat16
BF16NP = np.dtype(ml_dtypes.bfloat16)
RELU = mybir.ActivationFunctionType.Relu
COPY = mybir.ActivationFunctionType.Copy


@with_exitstack
def _tile_kernel(ctx, tc, outs, ins, ngroups):
    nc = tc.nc
    xpk, w_s, w0, w1, bvec, ident_d, ones_d = ins
    (out_d,) = outs

    const = ctx.enter_context(tc.tile_pool(name="const", bufs=1))
    xpool = ctx.enter_context(tc.tile_pool(name="xp", bufs=LOOKAHEAD + 1))
    spool = ctx.enter_context(tc.tile_pool(name="sp", bufs=3))
    opool = ctx.enter_context(tc.tile_pool(name="op", bufs=3))
    ppool = ctx.enter_context(tc.tile_pool(name="ps", bufs=2, space="PSUM"))
    pout = ctx.enter_context(tc.tile_pool(name="po", bufs=2, space="PSUM"))

    split = {0, ngroups - 1}  # groups DMA'd in 4 pieces for short ramp/tail

    def issue_loads(g):
        """Every group rides both HWDGE rings: xn0+xs on SP, xn1 on ACT
        (stores join ACT).  Ramp/tail groups split each half again so the
        first fold starts earlier / the serial tail is shorter."""
        r = slice(g * GROUP, (g + 1) * GROUP)
        t = xpool.tile([128, PACK], BF16, tag="t")
        if g in split:
            nc.sync.dma_start(t[:, 0:2048], xpk[r, 0:2048])
            nc.scalar.dma_start(t[:, T1B:T1B + 2048], xpk[r, T1B:T1B + 2048])
            nc.sync.dma_start(t[:, 2048:T1B], xpk[r, 2048:T1B])
            nc.scalar.dma_start(t[:, T1B + 2048:PACK],
                                xpk[r, T1B + 2048:PACK])
        else:
            nc.sync.dma_start(t[:, 0:T1B], xpk[r, 0:T1B])
            nc.scalar.dma_start(t[:, T1B:PACK], xpk[r, T1B:PACK])
        return t

    pending = [issue_loads(0)]

    ident = const.tile([128, 128], BF16, tag="ident")
    nc.sync.dma_start(ident[:], ident_d[:])
    wS_t = const.tile([128, HALF], BF16, tag="wS")
    nc.sync.dma_start(wS_t[:], w_s[:])
    w0_t = const.tile([128, HALF], BF16, tag="w0")
    nc.sync.dma_start(w0_t[:], w0[:])
    w1_t = const.tile([128, HALF], BF16, tag="w1")
    nc.sync.dma_start(w1_t[:], w1[:])
    b_t = const.tile([1, D], BF16, tag="b")
    nc.sync.dma_start(b_t[:], bvec[:])
    ones_t = const.tile([1, 128], BF16, tag="ones")
    nc.sync.dma_start(ones_t[:], ones_d[:])

    for g in range(1, min(LOOKAHEAD, ngroups)):
        pending.append(issue_loads(g))

    def fold(t, base, nslices, leave):
        """In-place binary-tree fold of `nslices` F-wide slices at column
        `base` down to `leave` slices; returns their column offsets.  The
        remaining slices are summed by PSUM-accumulating transposing
        matmuls, trading DVE time (the steady-state pacer) for PE time."""
        lv = nslices // 2
        while lv >= leave:
            nc.vector.tensor_add(t[:, base:base + lv * F],
                                 t[:, base:base + lv * F],
                                 t[:, base + lv * F:base + 2 * lv * F])
            lv //= 2
        return tuple(base + i * F for i in range(2 * lv))

    for g in range(ngroups):
        r = slice(g * GROUP, (g + 1) * GROUP)
        t = pending.pop(0)
        if g + LOOKAHEAD < ngroups:
            pending.append(issue_loads(g + LOOKAHEAD))

        # Fold each neighbour tensor's 32 slices to 2 (or 2x 16 -> 2+2 for
        # the split ramp/tail groups, whose halves arrive independently).
        if g in split:
            sl0 = fold(t, 0, 16, 2) + fold(t, 2048, 16, 2)
            sl1 = fold(t, T1B, 16, 2) + fold(t, T1B + 2048, 16, 2)
        else:
            sl0 = fold(t, 0, 32, 4)
            sl1 = fold(t, T1B, 32, 4)

        # pacc[:, 0:128] = sum_n x_n0 (as [f, bh]), [:, 128:256] = sum_n
        # x_n1, [:, 256:384] = x_self; accumulating transposing matmuls.
        pacc = ppool.tile([128, 3 * 128], F32, tag="pacc")
        nc.tensor.matmul(pacc[:, 256:384], t[:, NF:T1B], ident[:],
                         start=True, stop=True)
        for i, c in enumerate(sl0):
            nc.tensor.matmul(pacc[:, 0:128], t[:, c:c + F], ident[:],
                             start=(i == 0), stop=(i == len(sl0) - 1))
        for i, c in enumerate(sl1):
            nc.tensor.matmul(pacc[:, 128:256], t[:, c:c + F], ident[:],
                             start=(i == 0), stop=(i == len(sl1) - 1))

        # PSUM -> SBUF (bf16) in per-block copies so the self/t0 projections
        # don't wait for t1's transposes.
        sacc = spool.tile([128, 3 * 128], BF16, tag="sacc")
        nc.scalar.activation(sacc[:, 256:384], pacc[:, 256:384], COPY)
        nc.scalar.activation(sacc[:, 0:128], pacc[:, 0:128], COPY)
        nc.scalar.activation(sacc[:, 128:256], pacc[:, 128:256], COPY)

        # Projection: out[bh, d]; bias broadcast via K=1 matmuls, ordered so
        # the t1-dependent matmul is the only one on the critical tail.
        po = pout.tile([128, D], F32, tag="po")
        nc.tensor.matmul(po[:, 0:HALF], ones_t[:], b_t[:, 0:HALF],
                         start=True, stop=False)
        nc.tensor.matmul(po[:, 0:HALF], sacc[:, 256:384], wS_t[:],
                         start=False, stop=True)
        nc.tensor.matmul(po[:, HALF:D], ones_t[:], b_t[:, HALF:D],
                         start=True, stop=False)
        nc.tensor.matmul(po[:, HALF:D], sacc[:, 0:128], w0_t[:],
                         start=False, stop=False)
        nc.tensor.matmul(po[:, HALF:D], sacc[:, 128:256], w1_t[:],
                         start=False, stop=True)

        ob = opool.tile([128, D], BF16, tag="ob")
        nc.scalar.activation(ob[:], po[:], RELU)
        nc.scalar.dma_start(out_d[r, :], ob[:])


def build_nc(ngroups=NG):
    bh = ngroups * GROUP
    nc = bacc.Bacc("TRN2", target_bir_lowering=False, debug=False)
    xpk = nc.dram_tensor("xpk", [bh, PACK], BF16, kind="ExternalInput")
    w_s = nc.dram_tensor("w_s", [F, HALF], BF16, kind="ExternalInput")
    w0 = nc.dram_tensor("w0", [F, HALF], BF16, kind="ExternalInput")
    w1 = nc.dram_tensor("w1", [F, HALF], BF16, kind="ExternalInput")
    bvec = nc.dram_tensor("bvec", [1, D], BF16, kind="ExternalInput")
    ident_d = nc.dram_tensor("ident", [128, 128], BF16, kind="ExternalInput")
    ones_d = nc.dram_tensor("ones", [1, 128], BF16, kind="ExternalInput")
    out = nc.dram_tensor("out", [bh, D], BF16, kind="ExternalOutput")

    ins = [t.ap() for t in (xpk, w_s, w0, w1, bvec, ident_d, ones_d)]
    with nc.allow_low_precision("2e-2 rel-err budget admits bf16 datapath"):
        with tile.TileContext(nc) as tc:
            _tile_kernel(tc, [out.ap()], ins, ngroups)
    nc.compile()
    return nc


def make_in_maps(x_self, x_neigh_0, x_neigh_1, w_self, w_neigh_0, w_neigh_1, b):
    """Shard full inputs into per-core input maps (batch axis, 8 ways).

    All operands are cast to bf16 here (host side): the 2e-2 tolerance
    admits it and it halves the HBM traffic of this memory-bound kernel.
    xn0/xn1/xs are packed into one row-major tensor so each row group is
    a single large DMA.
    """
    xs16 = np.asarray(x_self, dtype=np.float32).astype(BF16NP)
    xn0_16 = np.asarray(x_neigh_0, dtype=np.float32).astype(BF16NP)
    xn1_16 = np.asarray(x_neigh_1, dtype=np.float32).astype(BF16NP)
    scale = np.float32(1.0 / (N * NR))
    w_s = np.asarray(w_self, dtype=np.float32).astype(BF16NP)
    w0 = (np.asarray(w_neigh_0, dtype=np.float32) * scale).astype(BF16NP)
    w1 = (np.asarray(w_neigh_1, dtype=np.float32) * scale).astype(BF16NP)
    bvec = np.asarray(b, dtype=np.float32).reshape(1, D).astype(BF16NP)
    ident = np.eye(128, dtype=np.float32).astype(BF16NP)
    ones = np.ones((1, 128), dtype=np.float32).astype(BF16NP)

    xpk = np.empty((B * H, PACK), dtype=BF16NP)
    xpk[:, 0:NF] = xn0_16.reshape(B * H, NF)
    xpk[:, NF:T1B] = xs16.reshape(B * H, F)
    xpk[:, T1B:PACK] = xn1_16.reshape(B * H, NF)

    in_maps = []
    for c in range(NCORES):
        rs = slice(c * BH, (c + 1) * BH)
        in_maps.append({
            "xpk": np.ascontiguousarray(xpk[rs]),
            "w_s": w_s, "w0": w0, "w1": w1, "bvec": bvec,
            "ident": ident, "ones": ones,
        })
    return in_maps


_NC_CACHE = None


def kernel(x_self, x_neigh_0, x_neigh_1, w_self, w_neigh_0, w_neigh_1, b):
    global _NC_CACHE
    if _NC_CACHE is None:
        _NC_CACHE = build_nc()
    in_maps = make_in_maps(x_self, x_neigh_0, x_neigh_1,
                           w_self, w_neigh_0, w_neigh_1, b)
    res = bass_utils.run_bass_kernel_spmd(
        _NC_CACHE, in_maps, core_ids=list(range(NCORES)))
    out = np.concatenate([r["out"] for r in res.results], axis=0)
    return out.astype(np.float32).reshape(B, H, D)
